# revision 29
# baseline (speedup 1.0000x reference)
"""GRU-D layer kernel for Trainium2, 8 NeuronCores, batch-parallel.

Problem shapes: x [256, 512, 128], h_decay [256, 512], H=256.
Sharding: batch 256 -> 32 per core; GRU weights replicated.

Per-core device layout (all recurrence tensors transposed: [H-chunks on
128 partitions, batch on free dim]):
  hT [128, 64] : col j = (chunk c=j//32, batch b=j%32), h-index = 128*c + p.

Phase 1: proj = x @ [Wz|Wr|Wh] + bias  (PE GEMM, bf16 output, SBUF-resident
         per quarter of T, interleaved with phase 2 of previous quarter).
Phase 2: per step t:
  h_dec = dec_t * h            (DVE, decb shipped pre-broadcast from host)
  zr/h preactivations = proj_t (folded in via identity-rhs matmuls)
                        + U @ h_dec (U stationary bf16 blocks)
  r = sigmoid(psum_r); z = sigmoid(psum_z)
  h_prop = tanh(psum_h with (r*h_dec) @ Uh)
  h_new = z*h_prop - (z-1)*dec*h   [= z*h_prop + (1-z)*h_dec]
"""

import numpy as np

B, T, D, H = 256, 512, 128, 256
NCORES = 8
BS = B // NCORES  # 32

TRACE = False
LAST_EXEC_NS = None

_NC_CACHE = {}


def _build(T_steps, variant=()):
    vset = set(variant)
    import concourse.bass as bass
    import concourse.mybir as mybir
    from concourse.tile import TileContext

    f32 = mybir.dt.float32
    bf16 = mybir.dt.bfloat16
    SIG = mybir.ActivationFunctionType.Sigmoid
    TANH = mybir.ActivationFunctionType.Tanh
    CPY = mybir.ActivationFunctionType.Copy
    MUL = mybir.AluOpType.mult
    SUB = mybir.AluOpType.subtract

    NT = T_steps // 4        # 4 timesteps per x-tile (128 bt rows)
    NQ = 4 if T_steps >= 16 else 1
    QT = T_steps // NQ       # steps per quarter
    QNT = NT // NQ           # x-tiles per quarter
    GS = 16 if T_steps >= 16 else T_steps  # steps per output/dec DMA group
    NG = T_steps // GS

    nc = bass.Bass()
    xT_d = nc.dram_tensor("xT", [NT, 128, 128], f32, kind="ExternalInput")
    W_d = nc.dram_tensor("W", [128, 768], f32, kind="ExternalInput")
    b3_d = nc.dram_tensor("b3", [1, 768], f32, kind="ExternalInput")
    ones_d = nc.dram_tensor("ones", [1, 128], f32, kind="ExternalInput")
    uzr_d = nc.dram_tensor("Uzr", [8, 128, 128], bf16, kind="ExternalInput")
    uh_d = nc.dram_tensor("Uh4", [4, 128, 128], bf16, kind="ExternalInput")
    i32_d = nc.dram_tensor("I32", [128, 32], bf16, kind="ExternalInput")
    # decb/outG are laid out exactly like their SBUF tiles: [group, 128
    # partitions, GS*64] with col = 64*t_local + (chunk*32 + batch).
    decb_d = nc.dram_tensor("decb", [NG, 128, GS * 64], f32,
                            kind="ExternalInput")
    outG_d = nc.dram_tensor("outG", [NG, 128, GS * 64], f32,
                            kind="ExternalOutput")

    with TileContext(nc) as tc:
        with (
            tc.tile_pool(name="res", bufs=1) as res,
            tc.tile_pool(name="projp", bufs=2) as projp,
            tc.tile_pool(name="x1", bufs=3) as x1,
            tc.tile_pool(name="p1ps", bufs=1, space="PSUM") as p1ps,
            tc.tile_pool(name="pz", bufs=2, space="PSUM") as pzp,
            tc.tile_pool(name="pr", bufs=2, space="PSUM") as prp,
            tc.tile_pool(name="ph", bufs=2, space="PSUM") as php,
            tc.tile_pool(name="hp", bufs=3) as hpool,
            tc.tile_pool(name="wk", bufs=3) as wk,
            tc.tile_pool(name="dec", bufs=4) as decp,
        ):
            # ---- resident constants ----
            w_sb = res.tile([128, 768], f32)
            nc.sync.dma_start(out=w_sb, in_=W_d[:])
            b3_sb = res.tile([1, 768], f32)
            nc.sync.dma_start(out=b3_sb, in_=b3_d[:])
            ones_sb = res.tile([1, 128], f32)
            nc.sync.dma_start(out=ones_sb, in_=ones_d[:])
            uzr = res.tile([128, 1024], bf16)
            nc.sync.dma_start(
                out=uzr[:].rearrange("p (i m) -> p i m", i=8),
                in_=uzr_d.rearrange("i p m -> p i m"),
            )
            uh = res.tile([128, 512], bf16)
            nc.sync.dma_start(
                out=uh[:].rearrange("p (i m) -> p i m", i=4),
                in_=uh_d.rearrange("i p m -> p i m"),
            )
            i32 = res.tile([128, 32], bf16)
            nc.sync.dma_start(out=i32, in_=i32_d[:])

            # ---- phase 1: one x-tile -> 768 cols of proj for 4 timesteps ----
            def p1_tile(proj_buf, q, j):
                if "no_p1" in vset:
                    return
                k = q * QNT + j
                xt = x1.tile([128, 128], f32, tag="xt")
                nc.sync.dma_start(out=xt, in_=xT_d[k])
                ps = p1ps.tile([128, 768], f32, tag="p1")
                nc.tensor.matmul(ps[:, 0:512], ones_sb[:], b3_sb[:, 0:512],
                                 start=True, stop=False)
                nc.tensor.matmul(ps[:, 512:768], ones_sb[:], b3_sb[:, 512:768],
                                 start=True, stop=False)
                nc.tensor.matmul(ps[:, 0:512], xt[:], w_sb[:, 0:512],
                                 start=False, stop=True)
                nc.tensor.matmul(ps[:, 512:768], xt[:], w_sb[:, 512:768],
                                 start=False, stop=True)
                nc.scalar.activation(out=proj_buf[:, j * 768:(j + 1) * 768],
                                     in_=ps[:], func=CPY)

            # ---- phase 2 helpers ----
            def proj_sl(proj_buf, tl, lo, hi):
                j, ts = tl // 4, tl % 4
                return proj_buf[32 * ts:32 * ts + 32, j * 768 + lo:j * 768 + hi]

            def fold_kw(tl):
                ts = tl % 4
                return ({"tile_position": (96, 0)} if ts == 3 else {},
                        i32[32 * ts:32 * ts + 32, :])

            def emit_fold_r(proj_buf, tl):
                kw, rhs = fold_kw(tl)
                pr = prp.tile([128, 64], f32, tag="pr")
                if "no_fold" in vset:
                    return pr
                nc.tensor.matmul(pr[:, 0:32], proj_sl(proj_buf, tl, 256, 384),
                                 rhs, start=True, stop=False, **kw)
                nc.tensor.matmul(pr[:, 32:64], proj_sl(proj_buf, tl, 384, 512),
                                 rhs, start=False, stop=False, **kw)
                return pr

            def emit_fold_z(proj_buf, tl):
                kw, rhs = fold_kw(tl)
                pz = pzp.tile([128, 64], f32, tag="pz")
                if "no_fold" in vset:
                    return pz
                nc.tensor.matmul(pz[:, 0:32], proj_sl(proj_buf, tl, 0, 128),
                                 rhs, start=True, stop=False, **kw)
                nc.tensor.matmul(pz[:, 32:64], proj_sl(proj_buf, tl, 128, 256),
                                 rhs, start=False, stop=False, **kw)
                return pz

            def emit_fold_h(proj_buf, tl):
                kw, rhs = fold_kw(tl)
                ph = php.tile([128, 64], f32, tag="ph")
                if "no_fold" in vset:
                    return ph
                nc.tensor.matmul(ph[:, 0:32], proj_sl(proj_buf, tl, 512, 640),
                                 rhs, start=True, stop=False, **kw)
                nc.tensor.matmul(ph[:, 32:64], proj_sl(proj_buf, tl, 640, 768),
                                 rhs, start=False, stop=False, **kw)
                return ph

            def emit_folds(proj_buf, tl):
                """Identity-rhs matmuls placing proj_t (transposed) into the
                step's psum banks; first matmul per bank clears it."""
                pr = emit_fold_r(proj_buf, tl)
                pz = emit_fold_z(proj_buf, tl)
                ph = emit_fold_h(proj_buf, tl)
                return pz, pr, ph

            # ---- prologue ----
            proj_cur = projp.tile([128, QNT * 768], bf16, tag="proj")
            for j in range(QNT):
                p1_tile(proj_cur, 0, j)

            h0 = res.tile([128, 64], f32)
            nc.any.memzero(h0)
            h_prev = h0[:]

            def load_decg(g):
                dt_ = decp.tile([128, GS * 64], f32, tag="db")
                if "no_decb" not in vset:
                    nc.sync.dma_start(out=dt_, in_=decb_d[g])
                else:
                    nc.any.memzero(dt_)
                return dt_

            decg = load_decg(0)
            decg_nxt = None
            hgrp = None

            # ---- main loop ----
            for q in range(NQ):
                proj_nxt = None
                if q + 1 < NQ:
                    proj_nxt = projp.tile([128, QNT * 768], bf16, tag="proj")
                pending = emit_folds(proj_cur, 0)
                for tl in range(QT):
                    t = q * QT + tl
                    g, tg = t // GS, t % GS
                    pz, pr, ph = pending

                    if tg == 0:
                        if g + 1 < NG:
                            decg_nxt = load_decg(g + 1)
                        hgrp = hpool.tile([128, GS * 64], f32, tag="hh")
                    db = decg[:, 64 * tg:64 * tg + 64]

                    hdec = wk.tile([128, 64], bf16, tag="hdec")
                    if "no_dve" not in vset:
                        nc.vector.tensor_tensor(out=hdec, in0=h_prev,
                                                in1=db, op=MUL)

                    # r then z gate accumulation (r first: it gates h_prop)
                    if "no_umm" not in vset:
                        for mc in range(2):
                            for kc in range(2):
                                i0 = ((2 + mc) * 2 + kc) * 128
                                nc.tensor.matmul(
                                    pr[:, 32 * mc:32 * mc + 32],
                                    uzr[:, i0:i0 + 128],
                                    hdec[:, 32 * kc:32 * kc + 32],
                                    start=False, stop=(mc == 1 and kc == 1))
                        for mc in range(2):
                            for kc in range(2):
                                i0 = (mc * 2 + kc) * 128
                                nc.tensor.matmul(
                                    pz[:, 32 * mc:32 * mc + 32],
                                    uzr[:, i0:i0 + 128],
                                    hdec[:, 32 * kc:32 * kc + 32],
                                    start=False, stop=(mc == 1 and kc == 1))

                    # next step's folds fill the PE gap while ACT/DVE work
                    pesched = "pesched" in vset
                    if tl + 1 < QT and not pesched:
                        pending = emit_folds(proj_cur, tl + 1)
                    if tl + 1 < QT and pesched:
                        ph_n = emit_fold_h(proj_cur, tl + 1)
                        pz_n = emit_fold_z(proj_cur, tl + 1)

                    # interleave next quarter's projection GEMM
                    if proj_nxt is not None and tl % 4 == 0 and tl // 4 < QNT:
                        p1_tile(proj_nxt, q + 1, tl // 4)

                    r_bf = wk.tile([128, 64], bf16, tag="rbf")
                    z_s = wk.tile([128, 64], f32, tag="zs")
                    if "no_act" not in vset:
                        nc.scalar.activation(out=r_bf, in_=pr[:], func=SIG)
                        nc.scalar.activation(out=z_s, in_=pz[:], func=SIG)

                    rh = wk.tile([128, 64], bf16, tag="rh")
                    if "no_dve" not in vset:
                        nc.vector.tensor_tensor(out=rh, in0=r_bf[:],
                                                in1=hdec[:], op=MUL)

                    if "no_umm" not in vset:
                        for mc in range(2):
                            for kc in range(2):
                                i0 = (mc * 2 + kc) * 128
                                nc.tensor.matmul(
                                    ph[:, 32 * mc:32 * mc + 32],
                                    uh[:, i0:i0 + 128],
                                    rh[:, 32 * kc:32 * kc + 32],
                                    start=False, stop=(mc == 1 and kc == 1))

                    if tl + 1 < QT and pesched:
                        pr_n = emit_fold_r(proj_cur, tl + 1)
                        pending = (pz_n, pr_n, ph_n)

                    hp_t = wk.tile([128, 64], f32, tag="hpt")
                    if "no_act" not in vset:
                        nc.scalar.activation(out=hp_t, in_=ph[:], func=TANH)

                    # h_new = z*h_prop - (z-1)*dec*h_prev
                    eng = nc.gpsimd if "gps" in vset else nc.vector
                    a1 = wk.tile([128, 64], f32, tag="a1")
                    a2 = wk.tile([128, 64], f32, tag="a2")
                    b2 = wk.tile([128, 64], f32, tag="b2")
                    h_new = hgrp[:, 64 * tg:64 * tg + 64]
                    if "no_dve" not in vset:
                        eng.scalar_tensor_tensor(out=a1, in0=z_s[:],
                                                 scalar=1.0, in1=db,
                                                 op0=SUB, op1=MUL)
                        eng.tensor_tensor(out=a2, in0=a1[:], in1=h_prev,
                                          op=MUL)
                        nc.vector.tensor_tensor(out=b2, in0=z_s[:],
                                                in1=hp_t[:], op=MUL)
                        nc.vector.tensor_tensor(out=h_new, in0=b2[:],
                                                in1=a2[:], op=SUB)

                    if tg == GS - 1:
                        if "no_out" not in vset:
                            nc.sync.dma_start(out=outG_d[g], in_=hgrp[:])
                        decg = decg_nxt
                    h_prev = h_new
                proj_cur = proj_nxt

    _split_matmul_waits(nc, mybir)
    return nc


def _build2(T_steps, variant=()):
    """v2: chain-shortened recurrence.

    hdec_t = u_t - w'_t with
      u_t  = (dec_t * z_{t-1}) * hp_{t-1}     (DVE, right after tanh)
      w'_t = dec_t * (z_{t-1}-1) * hdec_{t-1} (Pool, off the tanh chain)
    r-gate psum accumulates xr + Ur@u + (-Ur)@w'; z-gate uses hdec directly.
    Critical chain/step: tanh -> DVE(u) -> PE(Ur@u) -> ACT(sigr) -> DVE(rh)
    -> PE(Uh@rh) -> ACT(tanh). Off-chain elementwise on Pool; phase-1 GEMM
    and proj copies in bf16 on PE/Pool.
    """
    vset = set(variant)
    import concourse.bass as bass
    import concourse.mybir as mybir
    from concourse.tile import TileContext

    f32 = mybir.dt.float32
    bf16 = mybir.dt.bfloat16
    SIG = mybir.ActivationFunctionType.Sigmoid
    TANH = mybir.ActivationFunctionType.Tanh
    MUL = mybir.AluOpType.mult
    SUB = mybir.AluOpType.subtract

    NT = T_steps // 4
    qsz = 64 if "nq8" in vset else 32
    NQ = max(4, T_steps // qsz) if T_steps >= 16 else 1
    QT = T_steps // NQ
    QNT = NT // NQ
    GS = (32 if "gs32" in vset else 16) if T_steps >= 32 else \
        (16 if T_steps >= 16 else T_steps)
    NG = T_steps // GS

    nc = bass.Bass()
    xT_d = nc.dram_tensor("xT", [NT, 128, 128], bf16, kind="ExternalInput")
    W_d = nc.dram_tensor("W", [128, 768], bf16, kind="ExternalInput")
    b3_d = nc.dram_tensor("b3", [1, 768], bf16, kind="ExternalInput")
    ones_d = nc.dram_tensor("ones", [1, 128], bf16, kind="ExternalInput")
    uzr_d = nc.dram_tensor("Uzr", [8, 128, 128], bf16, kind="ExternalInput")
    urn_d = nc.dram_tensor("Urn", [4, 128, 128], bf16, kind="ExternalInput")
    uh_d = nc.dram_tensor("Uh4", [4, 128, 128], bf16, kind="ExternalInput")
    i32_d = nc.dram_tensor("I32", [128, 32], bf16, kind="ExternalInput")
    decb_d = nc.dram_tensor("decb", [NG, 128, GS * 64], f32,
                            kind="ExternalInput")
    outG_d = nc.dram_tensor("outG", [NG, 128, GS * 64], f32,
                            kind="ExternalOutput")

    with TileContext(nc) as tc:
        with (
            tc.tile_pool(name="res", bufs=1) as res,
            tc.tile_pool(name="projp", bufs=2) as projp,
            tc.tile_pool(name="x1", bufs=3) as x1,
            tc.tile_pool(name="p1ps", bufs=1, space="PSUM") as p1ps,
            tc.tile_pool(name="pz", bufs=2, space="PSUM") as pzp,
            tc.tile_pool(name="pr", bufs=2, space="PSUM") as prp,
            tc.tile_pool(name="ph", bufs=2, space="PSUM") as php,
            tc.tile_pool(name="hp", bufs=3) as hpool,
            tc.tile_pool(name="wk", bufs=3) as wk,
            tc.tile_pool(name="uw", bufs=2) as uwp,
            tc.tile_pool(name="pk", bufs=3) as pk,
            tc.tile_pool(name="dec", bufs=4) as decp,
        ):
            # ---- resident constants ----
            w_sb = res.tile([128, 768], bf16)
            nc.sync.dma_start(out=w_sb, in_=W_d[:])
            b3_sb = res.tile([1, 768], bf16)
            nc.sync.dma_start(out=b3_sb, in_=b3_d[:])
            ones_sb = res.tile([1, 128], bf16)
            nc.sync.dma_start(out=ones_sb, in_=ones_d[:])
            uzr = res.tile([128, 1024], bf16)
            nc.sync.dma_start(
                out=uzr[:].rearrange("p (i m) -> p i m", i=8),
                in_=uzr_d.rearrange("i p m -> p i m"),
            )
            urn = res.tile([128, 512], bf16)
            nc.sync.dma_start(
                out=urn[:].rearrange("p (i m) -> p i m", i=4),
                in_=urn_d.rearrange("i p m -> p i m"),
            )
            uh = res.tile([128, 512], bf16)
            nc.sync.dma_start(
                out=uh[:].rearrange("p (i m) -> p i m", i=4),
                in_=uh_d.rearrange("i p m -> p i m"),
            )
            i32 = res.tile([128, 32], bf16)
            nc.sync.dma_start(out=i32, in_=i32_d[:])

            # ---- phase 1 ----
            CPY = mybir.ActivationFunctionType.Copy

            def p1_bias(q, j):
                """First half of a phase-1 tile: x DMA + bias matmuls."""
                k = q * QNT + j
                xt = x1.tile([128, 128], bf16, tag="xt")
                nc.sync.dma_start(out=xt, in_=xT_d[k])
                ps = p1ps.tile([128, 768], f32, tag="p1")
                nc.tensor.matmul(ps[:, 0:512], ones_sb[:], b3_sb[:, 0:512],
                                 start=True, stop=False)
                nc.tensor.matmul(ps[:, 512:768], ones_sb[:], b3_sb[:, 512:768],
                                 start=True, stop=False)
                return ps, xt

            def p1_wmm(ps, xt):
                nc.tensor.matmul(ps[:, 0:512], xt[:], w_sb[:, 0:512],
                                 start=False, stop=True)
                nc.tensor.matmul(ps[:, 512:768], xt[:], w_sb[:, 512:768],
                                 start=False, stop=True)

            def p1_tile(proj_buf, q, j, pieces=False):
                ps, xt = p1_bias(q, j)
                p1_wmm(ps, xt)
                if pieces:
                    return (ps, proj_buf, j)
                nc.scalar.activation(
                    out=proj_buf[:, j * 768:(j + 1) * 768], in_=ps[:],
                    func=CPY)
                return None

            def emit_copy_piece(pc, p):
                ps, proj_buf, j = pc
                lo = 192 * p
                dst = proj_buf[:, j * 768 + lo:j * 768 + lo + 192]
                src = ps[:, lo:lo + 192]
                if "ppool" in vset and p % 2 == 1:
                    nc.gpsimd.tensor_copy(out=dst, in_=src)
                else:
                    nc.scalar.activation(out=dst, in_=src, func=CPY)

            def proj_sl(proj_buf, tl, lo, hi):
                j, ts = tl // 4, tl % 4
                return proj_buf[32 * ts:32 * ts + 32, j * 768 + lo:j * 768 + hi]

            def fold_kw(tl):
                ts = tl % 4
                return ({"tile_position": (96, 0)} if ts == 3 else {},
                        i32[32 * ts:32 * ts + 32, :])

            def emit_pending(proj_buf, tl, wp):
                """Open next step's psum banks: xr fold + (-Ur)@w' into pr;
                xz fold into pz; xh fold into ph."""
                kw, rhs = fold_kw(tl)
                pr = prp.tile([128, 64], f32, tag="pr")
                nc.tensor.matmul(pr[:, 0:32], proj_sl(proj_buf, tl, 256, 384),
                                 rhs, start=True, stop=False, **kw)
                nc.tensor.matmul(pr[:, 32:64], proj_sl(proj_buf, tl, 384, 512),
                                 rhs, start=False, stop=False, **kw)
                for mc in range(2):
                    for kc in range(2):
                        i0 = (mc * 2 + kc) * 128
                        nc.tensor.matmul(
                            pr[:, 32 * mc:32 * mc + 32],
                            urn[:, i0:i0 + 128],
                            wp[:, 32 * kc:32 * kc + 32],
                            start=False, stop=False)
                pz = pzp.tile([128, 64], f32, tag="pz")
                nc.tensor.matmul(pz[:, 0:32], proj_sl(proj_buf, tl, 0, 128),
                                 rhs, start=True, stop=False, **kw)
                nc.tensor.matmul(pz[:, 32:64], proj_sl(proj_buf, tl, 128, 256),
                                 rhs, start=False, stop=False, **kw)
                ph = php.tile([128, 64], f32, tag="ph")
                nc.tensor.matmul(ph[:, 0:32], proj_sl(proj_buf, tl, 512, 640),
                                 rhs, start=True, stop=False, **kw)
                nc.tensor.matmul(ph[:, 32:64], proj_sl(proj_buf, tl, 640, 768),
                                 rhs, start=False, stop=False, **kw)
                return pz, pr, ph

            # ---- prologue ----
            proj_cur = projp.tile([128, QNT * 768], bf16, tag="proj")
            for j in range(QNT):
                p1_tile(proj_cur, 0, j)

            u_cur = uwp.tile([128, 64], bf16, tag="u")
            nc.any.memzero(u_cur)
            w_cur = uwp.tile([128, 64], bf16, tag="w")
            nc.any.memzero(w_cur)

            def load_decg(g):
                dt_ = decp.tile([128, GS * 64], f32, tag="db")
                nc.sync.dma_start(out=dt_, in_=decb_d[g])
                return dt_

            decg = load_decg(0)
            decg_nxt = None
            hgrp = None

            # ---- main loop ----
            pending = emit_pending(proj_cur, 0, w_cur)
            pcopy = None  # outstanding phase-1 psum awaiting piece copies
            for q in range(NQ):
                proj_nxt = None
                if q + 1 < NQ:
                    proj_nxt = projp.tile([128, QNT * 768], bf16, tag="proj")
                for tl in range(QT):
                    t = q * QT + tl
                    last = t == T_steps - 1
                    g, tg = t // GS, t % GS
                    pz, pr, ph = pending

                    if tg == 0:
                        if g + 1 < NG:
                            decg_nxt = load_decg(g + 1)
                        hgrp = hpool.tile([128, GS * 64], f32, tag="hh")
                    # dec_{t+1} slice (for u/w/c1); garbage-free at last step
                    if not last:
                        if tg + 1 < GS:
                            dbn = decg[:, 64 * (tg + 1):64 * (tg + 1) + 64]
                        else:
                            dbn = decg_nxt[:, 0:64]

                    # chain: Ur@u -> sigr
                    for mc in range(2):
                        for kc in range(2):
                            i0 = ((2 + mc) * 2 + kc) * 128
                            nc.tensor.matmul(
                                pr[:, 32 * mc:32 * mc + 32],
                                uzr[:, i0:i0 + 128],
                                u_cur[:, 32 * kc:32 * kc + 32],
                                start=False, stop=(mc == 1 and kc == 1))

                    hdec = wk.tile([128, 64], bf16, tag="hdec")
                    nc.vector.tensor_tensor(out=hdec, in0=u_cur[:],
                                            in1=w_cur[:], op=SUB)

                    # z-gate: Uz@hdec (off chain)
                    for mc in range(2):
                        for kc in range(2):
                            i0 = (mc * 2 + kc) * 128
                            nc.tensor.matmul(
                                pz[:, 32 * mc:32 * mc + 32],
                                uzr[:, i0:i0 + 128],
                                hdec[:, 32 * kc:32 * kc + 32],
                                start=False, stop=(mc == 1 and kc == 1))

                    r_bf = wk.tile([128, 64], bf16, tag="rbf")
                    nc.scalar.activation(out=r_bf, in_=pr[:], func=SIG)
                    z_s = wk.tile([128, 64], f32, tag="zs")
                    nc.scalar.activation(out=z_s, in_=pz[:], func=SIG)

                    rh = wk.tile([128, 64], bf16, tag="rh")
                    nc.vector.tensor_tensor(out=rh, in0=r_bf[:],
                                            in1=hdec[:], op=MUL)

                    # c1 (Pool) feeds the chain op u_{t+1}; g' = (z-1)*hdec
                    # on DVE (TensorScalarPtr is illegal on Pool); w' on Pool.
                    gp = pk.tile([128, 64], f32, tag="gp")
                    if not last:
                        c1 = pk.tile([128, 64], bf16, tag="c1")
                        nc.gpsimd.tensor_tensor(out=c1, in0=dbn,
                                                in1=z_s[:], op=MUL)
                    nc.vector.scalar_tensor_tensor(
                        out=gp, in0=z_s[:], scalar=1.0, in1=hdec[:],
                        op0=SUB, op1=MUL)
                    if not last:
                        w_nxt = uwp.tile([128, 64], bf16, tag="w")
                        nc.gpsimd.tensor_tensor(out=w_nxt, in0=dbn,
                                                in1=gp[:], op=MUL)

                    # interleave next quarter's projection GEMM: bias mms
                    # here (pre-Uh PE idle slot), W mms after pending
                    # (pre-Ur@u slot) -- each half fits the PE slack.
                    p1_pend = None
                    if proj_nxt is not None and tl % 4 == 0 and tl // 4 < QNT:
                        ps_xt = p1_bias(q + 1, tl // 4)
                        p1_pend = (ps_xt, proj_nxt, tl // 4)

                    # chain: Uh@rh -> tanh
                    for mc in range(2):
                        for kc in range(2):
                            i0 = (mc * 2 + kc) * 128
                            nc.tensor.matmul(
                                ph[:, 32 * mc:32 * mc + 32],
                                uh[:, i0:i0 + 128],
                                rh[:, 32 * kc:32 * kc + 32],
                                start=False, stop=(mc == 1 and kc == 1))

                    # open next step's banks while PE waits on nothing else
                    if not last:
                        ntl = tl + 1
                        if ntl < QT:
                            pending = emit_pending(proj_cur, ntl, w_nxt)
                        else:
                            pending = None  # emitted after quarter ends
                    if p1_pend is not None:
                        (ps, xt), pbuf, jj = p1_pend
                        p1_wmm(ps, xt)
                        pcopy = ((ps, pbuf, jj), 0)
                    hp_t = wk.tile([128, 64], bf16, tag="hpt")
                    nc.scalar.activation(out=hp_t, in_=ph[:], func=TANH)
                    if pcopy is not None:
                        pc, p = pcopy
                        emit_copy_piece(pc, p)
                        pcopy = (pc, p + 1) if p + 1 < 4 else None

                    if not last:
                        u_nxt = uwp.tile([128, 64], bf16, tag="u")
                        nc.vector.tensor_tensor(out=u_nxt, in0=c1[:],
                                                in1=hp_t[:], op=MUL)

                    # output blend on DVE: h = z*hp - g'
                    b2 = wk.tile([128, 64], f32, tag="b2")
                    nc.vector.tensor_tensor(out=b2, in0=z_s[:],
                                            in1=hp_t[:], op=MUL)
                    h_out = hgrp[:, 64 * tg:64 * tg + 64]
                    nc.vector.tensor_tensor(out=h_out, in0=b2[:],
                                            in1=gp[:], op=SUB)

                    if tg == GS - 1:
                        nc.sync.dma_start(out=outG_d[g], in_=hgrp[:])
                        decg = decg_nxt
                    if not last:
                        u_cur, w_cur = u_nxt, w_nxt
                # next quarter: emit pending for its first step
                proj_cur = proj_nxt
                if q + 1 < NQ:
                    pending = emit_pending(proj_cur, 0, w_cur)

    _split_matmul_waits(nc, mybir)
    return nc


def _split_matmul_waits(nc, mybir):
    """Walrus allows at most one sync wait per engine instruction. Move the
    excess onto same-engine NoOps inserted just before (avoids
    InstEventSemaphore, which is subject to the cayman event-accel
    deadlock).

    Keep the LAST-SATISFIED wait on the instruction itself (statically
    estimated from semaphore-update emission order): the early-satisfied
    waits retire on the NoOps long before, and the sequencer stalls in the
    wait queue at the real instruction, not at a NoOp that hides its
    decode latency."""
    # position at which each (sem id) reaches each cumulative value
    reach = {}
    count = {}
    pos = 0
    for func in nc.m.functions:
        for blk in func.blocks:
            for inst in blk.instructions:
                si = inst.sync_info
                if si is not None:
                    for upd in si.on_update:
                        sid = getattr(upd, "id", None)
                        if sid is None:
                            continue
                        val = getattr(upd, "update_value", 1) or 1
                        c = count.get(sid, 0)
                        for v in range(c + 1, c + val + 1):
                            reach.setdefault(sid, {})[v] = pos
                        count[sid] = c + val
                pos += 1

    def sat_pos(w):
        sid = getattr(w, "id", None)
        v = getattr(w, "wait_value", None)
        if sid is None or not isinstance(v, int):
            return 1 << 60  # unknown: treat as latest
        r = reach.get(sid, {})
        if v in r:
            return r[v]
        return 1 << 60

    for func in nc.m.functions:
        for blk in func.blocks:
            new_insts = []
            for inst in blk.instructions:
                si = inst.sync_info
                if si is not None and len(si.on_wait) > 1:
                    waits = list(si.on_wait)
                    keep_i = max(range(len(waits)),
                                 key=lambda i: (sat_pos(waits[i]), i))
                    for i, w in enumerate(waits):
                        if i == keep_i:
                            continue
                        nop = mybir.InstNoOp(
                            name=nc.get_next_instruction_name(),
                            sync_info=mybir.SyncInfo(on_wait=[w], on_update=[]),
                            engine=inst.engine,
                            bass_nofuse=True,
                        )
                        nc.register_instruction(nop)
                        new_insts.append(nop)
                    si.on_wait = [waits[keep_i]]
                new_insts.append(inst)
            blk.instructions[:] = new_insts


def _get_nc(T_steps=T, variant=()):
    key = (T_steps, tuple(variant))
    if key not in _NC_CACHE:
        if "v1" in variant:
            _NC_CACHE[key] = _build(
                T_steps, tuple(v for v in variant if v != "v1"))
        else:
            _NC_CACHE[key] = _build2(
                T_steps, tuple(v for v in variant if v != "v2"))
    return _NC_CACHE[key]


def _prep_shared(Wr, Wz, Wh, Ur, Uz, Uh, br, bz, bh):
    import ml_dtypes
    bf = ml_dtypes.bfloat16
    Wz, Wr, Wh = (np.asarray(a, np.float32) for a in (Wz, Wr, Wh))
    Uz, Ur, Uh = (np.asarray(a, np.float32) for a in (Uz, Ur, Uh))
    W = np.ascontiguousarray(np.concatenate([Wz, Wr, Wh], axis=1))
    b3 = np.ascontiguousarray(
        np.concatenate([np.asarray(bz), np.asarray(br), np.asarray(bh)])
        .reshape(1, 768).astype(np.float32))
    ones = np.ones((1, 128), np.float32)
    Uzr = np.empty((8, 128, 128), bf)
    for m in range(4):
        g = Uz if m < 2 else Ur
        mc = m % 2
        for kc in range(2):
            Uzr[m * 2 + kc] = g[128 * kc:128 * kc + 128,
                                128 * mc:128 * mc + 128].astype(bf)
    Uh4 = np.empty((4, 128, 128), bf)
    for mc in range(2):
        for kc in range(2):
            Uh4[mc * 2 + kc] = Uh[128 * kc:128 * kc + 128,
                                  128 * mc:128 * mc + 128].astype(bf)
    I32 = np.tile(np.eye(32, dtype=np.float32), (4, 1)).astype(bf)
    return dict(W=W, b3=b3, ones=ones, Uzr=Uzr, Uh4=Uh4, I32=I32)


def _prep_shared2(Wr, Wz, Wh, Ur, Uz, Uh, br, bz, bh):
    import ml_dtypes
    bf = ml_dtypes.bfloat16
    s = _prep_shared(Wr, Wz, Wh, Ur, Uz, Uh, br, bz, bh)
    Urn = np.empty((4, 128, 128), bf)
    Ur = np.asarray(Ur, np.float32)
    for mc in range(2):
        for kc in range(2):
            Urn[mc * 2 + kc] = (-Ur[128 * kc:128 * kc + 128,
                                    128 * mc:128 * mc + 128]).astype(bf)
    s["Urn"] = Urn
    for k in ("W", "b3", "ones"):
        s[k] = s[k].astype(bf)
    return s


def _gs_for(T_steps, variant=()):
    if T_steps >= 32 and "gs32" in variant:
        return 32
    return 16 if T_steps >= 16 else T_steps


def _prep_core(xs, ds, T_steps, ver=2, gs=None):
    # xs [32, T, 128] -> xT [T//4, 128d, 128bt]; col = 32*t_sub + b
    import ml_dtypes
    xs = np.asarray(xs, np.float32)
    ds = np.asarray(ds, np.float32)
    nt = T_steps // 4
    xr = xs.reshape(BS, nt, 4, 128).transpose(1, 3, 2, 0).reshape(nt, 128, 128)
    if ver == 2:
        xr = xr.astype(ml_dtypes.bfloat16)
    if gs is None:
        gs = 16 if T_steps >= 16 else T_steps
    ng = T_steps // gs
    # decb[g, p, 64*t' + 32*c + b] = ds[b, g*gs + t']  (independent of p, c)
    dT = ds.T.reshape(ng, gs, BS)                       # [g, t', b]
    db = np.concatenate([dT, dT], axis=2).reshape(ng, 1, gs * 64)
    decb = np.ascontiguousarray(
        np.broadcast_to(db, (ng, 128, gs * 64)).astype(np.float32))
    return dict(xT=np.ascontiguousarray(xr), decb=decb)


def _run_spmd(nc, in_maps, n_timed=0, n_trials=3):
    """Replicates bass2jax.run_bass_via_pjrt's multi-core path, optionally
    re-executing the compiled body with device-resident inputs to measure
    per-run wall time (no NTFF profiling hook exists in this environment)."""
    import time
    import jax
    import jax.numpy as jnp
    from jax.sharding import Mesh, PartitionSpec
    from jax.experimental.shard_map import shard_map
    import concourse.mybir as mybir
    from concourse import bass2jax
    from concourse.bass2jax import _bass_exec_p, partition_id_tensor

    bass2jax.install_neuronx_cc_hook()
    if not nc.is_finalized():
        nc.finalize()

    partition_name = (nc.partition_id_tensor.name
                      if nc.partition_id_tensor else None)
    in_names, out_names, out_avals, zero_outs = [], [], [], []
    for alloc in nc.m.functions[0].allocations:
        if not isinstance(alloc, mybir.MemoryLocationSet):
            continue
        name = alloc.memorylocations[0].name
        if alloc.kind == "ExternalInput":
            if name != partition_name:
                in_names.append(name)
        elif alloc.kind == "ExternalOutput":
            aval = jax.core.ShapedArray(
                tuple(alloc.tensor_shape), mybir.dt.np(alloc.dtype))
            out_names.append(name)
            out_avals.append(aval)
            zero_outs.append(np.zeros(aval.shape, aval.dtype))

    n_params = len(in_names)
    all_names = list(in_names) + list(out_names)
    if partition_name is not None:
        all_names.append(partition_name)

    def _body(*args):
        operands = list(args)
        if partition_name is not None:
            operands.append(partition_id_tensor())
        return tuple(_bass_exec_p.bind(
            *operands,
            out_avals=tuple(out_avals),
            in_names=tuple(all_names),
            out_names=tuple(out_names),
            lowering_input_output_aliases=(),
            sim_require_finite=True,
            sim_require_nnan=True,
            nc=nc,
        ))

    devices = jax.devices()[:NCORES]
    mesh = Mesh(np.asarray(devices), ("core",))
    nio = n_params + len(out_names)
    sharded = jax.jit(shard_map(
        _body, mesh=mesh,
        in_specs=(PartitionSpec("core"),) * nio,
        out_specs=(PartitionSpec("core"),) * len(out_names),
        check_rep=False), keep_unused=True)

    concat_in = [np.concatenate([np.asarray(m[name]) for m in in_maps], axis=0)
                 for name in in_names]
    concat_zeros = [np.zeros((NCORES * z.shape[0], *z.shape[1:]), z.dtype)
                    for z in zero_outs]
    args = concat_in + concat_zeros

    out_arrs = sharded(*args)
    jax.block_until_ready(out_arrs)

    times = []
    if n_timed:
        # Axon dispatch costs ~100ms per blocked round-trip, so time N
        # queued (unblocked) executions and difference totals: the device
        # runs them back-to-back.
        sharding = jax.sharding.NamedSharding(mesh, PartitionSpec("core"))
        dev_args = [jax.device_put(a, sharding) for a in args]
        jax.block_until_ready(dev_args)

        def _timed(n):
            t0 = time.perf_counter()
            o = None
            for _ in range(n):
                o = sharded(*dev_args)
            jax.block_until_ready(o)
            return time.perf_counter() - t0

        _timed(1)  # warm
        for _ in range(n_trials):
            t1 = _timed(1)
            tn = _timed(1 + n_timed)
            times.append((tn - t1) / n_timed)

    results = [
        {name: np.asarray(out_arrs[i]).reshape(NCORES, *out_avals[i].shape)[c]
         for i, name in enumerate(out_names)}
        for c in range(NCORES)
    ]
    return results, times


def _make_in_maps(x, h_decay, Wr, Wz, Wh, Ur, Uz, Uh, br, bz, bh, T_steps=T,
                  ver=2, variant=()):
    if ver == 2:
        shared = _prep_shared2(Wr, Wz, Wh, Ur, Uz, Uh, br, bz, bh)
    else:
        shared = _prep_shared(Wr, Wz, Wh, Ur, Uz, Uh, br, bz, bh)
    gs = _gs_for(T_steps, variant) if ver == 2 else None
    x = np.asarray(x, np.float32)
    h_decay = np.asarray(h_decay, np.float32)
    in_maps = []
    for c in range(NCORES):
        m = dict(shared)
        m.update(_prep_core(x[c * BS:(c + 1) * BS],
                            h_decay[c * BS:(c + 1) * BS], T_steps, ver=ver,
                            gs=gs))
        in_maps.append(m)
    return in_maps


def kernel(x, h_decay, Wr, Wz, Wh, Ur, Uz, Uh, br, bz, bh):
    global LAST_EXEC_NS
    nc = _get_nc(T)
    in_maps = _make_in_maps(x, h_decay, Wr, Wz, Wh, Ur, Uz, Uh, br, bz, bh)
    n_timed = 5 if TRACE else 0
    results, times = _run_spmd(nc, in_maps, n_timed=n_timed)
    if times:
        LAST_EXEC_NS = int(min(times) * 1e9)

    out = np.empty((B, T, H), np.float32)
    for c in range(NCORES):
        out[c * BS:(c + 1) * BS] = _unshard_out(results[c]["outG"], T)
    return out


def _unshard_out(oG, T_steps, gs=None):
    if gs is None:
        gs = 16 if T_steps >= 16 else T_steps
    ng = T_steps // gs
    # oG [g, p, 64t'+32c+b] -> [b, t, h=128c+p]
    o = oG.reshape(ng, 128, gs, 2, BS)          # [g, p, t', c, b]
    return o.transpose(4, 0, 2, 3, 1).reshape(BS, T_steps, H)

